# revision 1
# baseline (speedup 1.0000x reference)
"""Trainium2 Bass kernel for nn_BiDGNBlock (moe_routing).

Strategy: data-parallel over batch across 8 NeuronCores (no collectives —
measured collective floors on this stack are ~10-25us each, worse than
keeping the whole expert table local). Each core computes one batch element
end-to-end:
  - All fp32 constants are packed host-side into two blobs (2 DMAs total);
    the fp16 expert table We.T (8MB) streams in parallel on the SWDGE queue.
    A memset-sourced PE warm-up (HAM -> K=8/8) and ACT table preloads run
    during the DMA window.
  - BiMultiHeadAttention + layernorms + residuals in exact fp32 (the cosine
    router's top-2 picks have sim-gaps down to 4.4e-5, so this path must
    match the fp32 reference closely; fp32r/TF32 would flip routing picks).
  - CosineRouter: cosine sims via fp32 matmuls with host-normalized centers
    and on-device row norms; top-2 indices via the DVE max/max_index top-8
    instruction. The gate scalar (softmax over the top-2, summed) is exactly
    1.0, so only the indices matter.
  - Per-channel experts: the routed sum  moe[c] = sum_k out[c] @ We[tk].T
    is computed as  sum_e (mask_e * A.T)-matmuls accumulated in PSUM, where
    mask rows R.T[e,c] are built on-device from the indices and replicated
    across partitions via a small f16 DRAM round-trip. One fused DVE multiply
    masks 4 experts' activations at a time; 128 fp16 matmuls accumulate.
  - Final layernorm + residual in fp32 (beta+residual precombined while the
    PE drains the expert matmuls).
Measured: ~70.6us HW exec, rel err 1.7e-4 vs the fp32 reference.
"""

import sys
import numpy as np

sys.path.insert(0, "/opt/trn_rl_repo")

N_CORES = 8
B, C, T = 8, 64, 256
EXP = 32
KT = T // 128  # 2 k-tiles over the feature dim

_CACHE: dict = {}

# fp32 blob layouts: (name, partitions, shape). cols = prod(shape[1:]).
BLOB_A_SPEC = [
    ("xtl", 128, (128, KT, C)), ("xtr", 128, (128, KT, C)),
    ("wqt", 128, (128, KT, T)), ("wkt", 128, (128, KT, T)),
    ("wvt", 128, (128, KT, T)),
    ("bqp", 128, (128, KT)), ("bkp", 128, (128, KT)),
]
BLOB_B_SPEC = [
    ("wpt", 128, (128, KT, T)), ("wrt", 128, (128, 2 * KT, EXP)),
    ("ident", 128, (128, 128)), ("sel", 2, (2, 2, 128)),
    ("xl", 64, (64, T)), ("xr", 64, (64, T)),
    ("bv", 64, (64, T)), ("bp", 64, (64, T)),
    ("agl", 64, (64, T)), ("abl", 64, (64, T)),
    ("agr", 64, (64, T)), ("abr", 64, (64, T)),
    ("mgl", 64, (64, T)), ("mbl", 64, (64, T)),
    ("mgr", 64, (64, T)), ("mbr", 64, (64, T)),
    ("brp", 32, (32, 1)), ("cent", 32, (32, C)), ("eiota", 64, (64, 1)),
]


def _blob_layout():
    off = {}
    na = 0
    for name, parts, shape in BLOB_A_SPEC:
        cols = int(np.prod(shape[1:]))
        off[name] = (na, parts, shape)
        na += cols
    nb = 0
    for name, parts, shape in BLOB_B_SPEC:
        cols = int(np.prod(shape[1:]))
        off[name] = (nb, parts, shape)
        nb += cols
    return off, na, nb


BLOB_OFF, NA_COLS, NB_COLS = _blob_layout()


def _build():
    import concourse.bass as bass
    import concourse.mybir as mybir
    import concourse.tile as tile
    from concourse import bacc
    from contextlib import ExitStack

    dt = mybir.dt
    f32, f16 = dt.float32, dt.float16
    AF = mybir.ActivationFunctionType
    OP = mybir.AluOpType

    nc = bacc.Bacc("TRN2", target_bir_lowering=False, debug=False,
                   num_devices=N_CORES)

    def inp(name, shape, d=f32):
        return nc.dram_tensor(name, list(shape), d, kind="ExternalInput")

    vec_names = ["bv", "bp", "agl", "abl", "agr", "abr",
                 "mgl", "mbl", "mgr", "mbr"]
    # fp32 constants+inputs packed host-side into two blobs (one DMA each).
    # BLOB_SPEC: (name, partitions, shape) -- order shared with _prep_in_maps.
    blobA_d = inp("blobA", (128, NA_COLS))
    blobB_d = inp("blobB", (128, NB_COLS))
    weh_d = inp("weh", (128, C, KT, T), f16)  # We[e].T tiled [p, e, kt, u] fp16
    beh_d = inp("beh", (C, T), f16)           # be natural fp16

    ol2_d = nc.dram_tensor("ol2", [C, T], f32, kind="ExternalOutput")
    or2_d = nc.dram_tensor("or2", [C, T], f32, kind="ExternalOutput")

    with tile.TileContext(nc) as tc, ExitStack() as ctx:
        cst = ctx.enter_context(tc.tile_pool(name="cst", bufs=1))
        wk = ctx.enter_context(tc.tile_pool(name="wk", bufs=2))
        sm = ctx.enter_context(tc.tile_pool(name="sm", bufs=2))
        asc_p = ctx.enter_context(tc.tile_pool(name="asc", bufs=4))
        msk_p = ctx.enter_context(tc.tile_pool(name="msk", bufs=4))
        ps = ctx.enter_context(tc.tile_pool(name="ps", bufs=3, space="PSUM"))
        ps_moe_p = ctx.enter_context(tc.tile_pool(name="psmoe", bufs=1, space="PSUM"))

        def load(pool, d_tensor, shape, dty=f32, tag=None):
            t = pool.tile(list(shape), dty, tag=tag or d_tensor.name)
            nc.sync.dma_start(out=t, in_=d_tensor.ap())
            return t

        # ---- loads: blobA (attention-critical) first, then blobB, weh ----
        blobA = cst.tile([128, NA_COLS], f32, tag="blobA")
        nc.sync.dma_start(out=blobA, in_=blobA_d.ap())
        blobB = cst.tile([128, NB_COLS], f32, tag="blobB")
        nc.sync.dma_start(out=blobB, in_=blobB_d.ap())
        we_sb = cst.tile([128, C, KT, T], f16, tag="weh")
        wea = weh_d.ap()
        # same HWDGE queue as the blobs: FIFO order guarantees blobA/blobB
        # land first instead of competing with this 8MB bulk for SDMA engines
        for ch in range(4):
            nc.sync.dma_start(out=we_sb[:, ch * 16:(ch + 1) * 16],
                              in_=wea[:, ch * 16:(ch + 1) * 16])
        beh = cst.tile([C, T], f16, tag="beh")
        nc.sync.dma_start(out=beh, in_=beh_d.ap())

        def bview(blob, name):
            off, parts, shape = BLOB_OFF[name]
            cols = 1
            for s in shape[1:]:
                cols *= s
            v = blob[0:parts, off:off + cols]
            if len(shape) == 3:
                v = v.rearrange("p (a b) -> p a b", a=shape[1])
            return v

        xtl = bview(blobA, "xtl")
        xtr = bview(blobA, "xtr")
        wqt = bview(blobA, "wqt")
        wkt = bview(blobA, "wkt")
        wvt = bview(blobA, "wvt")
        bqp = bview(blobA, "bqp")
        bkp = bview(blobA, "bkp")
        wpt = bview(blobB, "wpt")
        wrt = bview(blobB, "wrt")
        ident = bview(blobB, "ident")
        sel = bview(blobB, "sel")
        xl_sb = bview(blobB, "xl")
        xr_sb = bview(blobB, "xr")
        vec = {n: bview(blobB, n) for n in vec_names}
        brp = bview(blobB, "brp")
        cent = bview(blobB, "cent")
        eiota = bview(blobB, "eiota")

        eps_t = cst.tile([C, 1], f32, tag="eps")
        nc.vector.memset(eps_t, 1e-5)

        # PE warm-up from a memset tile (no DMA dependency): HAM reaches
        # K=8/8 during the input DMA window. Also preload the ACT tables.
        warm_p = ctx.enter_context(tc.tile_pool(name="warm", bufs=1, space="PSUM"))
        wsrc = cst.tile([128, 512], f16, tag="wsrc")
        nc.vector.memset(wsrc, 0.5)
        pw = warm_p.tile([128, 512], f32, tag="warm")
        for wi in range(14):
            nc.tensor.matmul(pw, wsrc[:, 0:128], wsrc,
                             start=True, stop=True, skip_group_check=True)
        wact = cst.tile([1, 32], f32, tag="wact")
        nc.vector.memset(wact, 1.0)
        nc.scalar.activation(out=wact, in_=wact, func=AF.Exp)
        nc.scalar.activation(out=wact, in_=wact, func=AF.Sqrt)

        # ---- attention: q.T, k.T ----
        qt = wk.tile([128, KT, C], f32, tag="qt")
        ktl = wk.tile([128, KT, C], f32, tag="ktl")
        for (src, w, bias, dst) in [(xtl, wqt, bqp, qt), (xtr, wkt, bkp, ktl)]:
            for ut in range(KT):
                p = ps.tile([128, C], f32, tag="ps")
                for kt in range(KT):
                    nc.tensor.matmul(p, w[:, kt, ut * 128:(ut + 1) * 128],
                                     src[:, kt], start=(kt == 0), stop=(kt == KT - 1))
                nc.vector.tensor_scalar(out=dst[:, ut], in0=p,
                                        scalar1=bias[:, ut:ut + 1], scalar2=None,
                                        op0=OP.add)

        # ---- v = (x_l - x_r) @ Wv.T + bv  (natural layout [c, u]) ----
        xdt = wk.tile([128, KT, C], f32, tag="xdt")
        nc.vector.tensor_sub(xdt, xtl, xtr)
        pv = ps.tile([C, T], f32, tag="ps")
        for kt in range(KT):
            nc.tensor.matmul(pv, xdt[:, kt], wvt[:, kt],
                             start=(kt == 0), stop=(kt == KT - 1))
        v_sb = wk.tile([C, T], f32, tag="v")
        nc.vector.tensor_tensor(out=v_sb, in0=pv, in1=vec["bv"], op=OP.add)

        # ---- energy + softmax ----
        pe_ = ps.tile([C, C], f32, tag="ps")
        for ut in range(KT):
            nc.tensor.matmul(pe_, qt[:, ut], ktl[:, ut],
                             start=(ut == 0), stop=(ut == KT - 1))
        rowmax = sm.tile([C, 1], f32, tag="rowmax")
        nc.vector.tensor_reduce(rowmax, pe_, axis=mybir.AxisListType.X, op=OP.max)
        attn = wk.tile([C, C], f32, tag="attn")
        nc.vector.tensor_scalar(out=attn, in0=pe_, scalar1=rowmax, scalar2=1.0 / 16.0,
                                op0=OP.subtract, op1=OP.mult)
        nc.scalar.activation(out=attn, in_=attn, func=AF.Exp)
        rowsum = sm.tile([C, 1], f32, tag="rowsum")
        nc.vector.tensor_reduce(rowsum, attn, axis=mybir.AxisListType.X, op=OP.add)
        nc.vector.reciprocal(rowsum, rowsum)
        nc.vector.tensor_scalar_mul(attn, attn, rowsum)

        # ---- attn.T ----
        pat = ps.tile([C, C], f32, tag="ps")
        nc.tensor.transpose(pat, attn, ident[0:C, 0:C])
        attnT = wk.tile([C, C], f32, tag="attnT")
        nc.vector.tensor_copy(attnT, pat)

        # ---- out_l.T / out_r.T  [u, c] ----
        oLT = wk.tile([128, KT, C], f32, tag="oLT")
        oRT = wk.tile([128, KT, C], f32, tag="oRT")
        for ut in range(KT):
            pl = ps.tile([128, C], f32, tag="ps")
            nc.tensor.matmul(pl, v_sb[:, ut * 128:(ut + 1) * 128], attnT,
                             start=True, stop=True)
            nc.vector.tensor_copy(oLT[:, ut], pl)
            pr = ps.tile([128, C], f32, tag="ps")
            nc.tensor.matmul(pr, v_sb[:, ut * 128:(ut + 1) * 128], attn,
                             start=True, stop=True)
            nc.vector.tensor_copy(oRT[:, ut], pr)

        # ---- proj + LN + residual -> OUT_L / OUT_R (natural [c, u]) ----
        def ln_block(src_ps, bias_t, gamma, beta, resid, out_tile, stag):
            if bias_t is not None:
                nc.vector.tensor_tensor(out=out_tile, in0=src_ps, in1=bias_t,
                                        op=OP.add)
            else:
                nc.vector.tensor_copy(out_tile, src_ps)
            stats = sm.tile([C, 6], f32, tag="stats" + stag)
            nc.vector.bn_stats(out=stats, in_=out_tile)
            mv = sm.tile([C, 2], f32, tag="mv" + stag)
            nc.vector.bn_aggr(out=mv, in_=stats)
            rstd = sm.tile([C, 1], f32, tag="rstd" + stag)
            nc.scalar.activation(out=rstd, in_=mv[:, 1:2], func=AF.Sqrt,
                                 bias=eps_t)
            nc.vector.reciprocal(rstd, rstd)
            nc.vector.tensor_scalar(out=out_tile, in0=out_tile,
                                    scalar1=mv[:, 0:1], scalar2=rstd,
                                    op0=OP.subtract, op1=OP.mult)
            nc.vector.tensor_tensor(out=out_tile, in0=out_tile, in1=gamma,
                                    op=OP.mult)
            nc.vector.tensor_tensor(out=out_tile, in0=out_tile, in1=beta,
                                    op=OP.add)
            nc.vector.tensor_tensor(out=out_tile, in0=out_tile, in1=resid,
                                    op=OP.add)

        OUT_L = wk.tile([C, T], f32, tag="OUTL")
        OUT_R = wk.tile([C, T], f32, tag="OUTR")
        for (oT, g, bt, resid, out_t) in [
                (oLT, vec["agl"], vec["abl"], xl_sb, OUT_L),
                (oRT, vec["agr"], vec["abr"], xr_sb, OUT_R)]:
            pp = ps.tile([C, T], f32, tag="ps")
            for ut in range(KT):
                nc.tensor.matmul(pp, oT[:, ut], wpt[:, ut],
                                 start=(ut == 0), stop=(ut == KT - 1))
            ln_block(pp, vec["bp"], g, bt, resid, out_t, "1")

        # ---- transposes of OUT_L/OUT_R -> [u(128), kt, c] tiles ----
        oLT2 = wk.tile([128, KT, C], f32, tag="oLT2")
        oRT2 = wk.tile([128, KT, C], f32, tag="oRT2")
        for (src, dst) in [(OUT_L, oLT2), (OUT_R, oRT2)]:
            for ut in range(KT):
                pt = ps.tile([128, C], f32, tag="ps")
                nc.tensor.transpose(pt, src[:, ut * 128:(ut + 1) * 128],
                                    ident[0:C, 0:C])
                nc.vector.tensor_copy(dst[:, ut], pt)

        # ---- router ----
        rtiles = [(oLT2, 0), (oLT2, 1), (oRT2, 0), (oRT2, 1)]
        # xp.T [d, c] (for the sim matmul's contraction over d)
        pxp = ps.tile([EXP, C], f32, tag="ps")
        for j, (tl, kt) in enumerate(rtiles):
            nc.tensor.matmul(pxp, wrt[:, j], tl[:, kt],
                             start=(j == 0), stop=(j == 3))
        xpT = wk.tile([EXP, C], f32, tag="xpT")
        nc.vector.tensor_scalar(out=xpT, in0=pxp, scalar1=brp, scalar2=None,
                                op0=OP.add)
        # xp natural [c, d] via PE transpose of xpT (for the row norms)
        pxn = ps.tile([C, EXP], f32, tag="ps")
        nc.tensor.transpose(pxn, xpT, ident[0:EXP, 0:EXP])
        xpn = sm.tile([C, EXP], f32, tag="xpn")
        nc.vector.tensor_copy(xpn, pxn)

        sq = sm.tile([C, EXP], f32, tag="sq")
        nc.vector.tensor_mul(sq, xpn, xpn)
        ssum = sm.tile([C, 1], f32, tag="ssum")
        nc.vector.tensor_reduce(ssum, sq, axis=mybir.AxisListType.X, op=OP.add)
        nc.scalar.activation(out=ssum, in_=ssum, func=AF.Sqrt)
        nc.vector.tensor_scalar_max(ssum, ssum, 1e-12)
        nc.vector.reciprocal(ssum, ssum)

        psim = ps.tile([C, C], f32, tag="ps")
        nc.tensor.matmul(psim, xpT, cent, start=True, stop=True)
        sim_sb = wk.tile([C, C], f32, tag="sim")
        nc.vector.tensor_scalar_mul(sim_sb, psim, ssum)

        mx8 = sm.tile([C, 8], f32, tag="mx8")
        nc.vector.max(out=mx8, in_=sim_sb)
        idx8 = sm.tile([C, 8], mybir.dt.uint32, tag="idx8")
        nc.vector.max_index(out=idx8, in_max=mx8, in_values=sim_sb)
        topif = sm.tile([C, 2], f32, tag="topif")
        nc.vector.tensor_copy(topif, idx8[:, 0:2])

        # ---- replicate topi rows across all 128 partitions via PE ----
        ptt = ps.tile([2, C], f32, tag="ps")
        nc.tensor.transpose(ptt, topif, ident[0:C, 0:C])
        ttT = sm.tile([2, C], f32, tag="ttT")
        nc.vector.tensor_copy(ttT, ptt)
        ttrep_ps = []
        for k in range(2):
            pr = ps.tile([128, C], f32, tag="ps")
            nc.tensor.matmul(pr, sel[:, k], ttT, start=True, stop=True)
            ttrep_ps.append(pr)

        # R.T[e, c] for the bias matmul (fp16)
        RT = wk.tile([C, C], f32, tag="RT")
        RT1 = sm.tile([C, C], f32, tag="RT1")
        nc.vector.tensor_scalar(out=RT, in0=ttrep_ps[0][0:C], scalar1=eiota,
                                scalar2=None, op0=OP.is_equal)
        nc.vector.tensor_scalar(out=RT1, in0=ttrep_ps[1][0:C], scalar1=eiota,
                                scalar2=None, op0=OP.is_equal)
        nc.vector.tensor_add(RT, RT, RT1)
        RTh = wk.tile([C, C], f16, tag="RTh")
        nc.vector.tensor_copy(RTh, RT)

        # ---- fp16 copy of transposed activations [p, kt, side, c] ----
        oAll = wk.tile([128, KT, 2, C], f16, tag="oAll")
        for kt in range(KT):
            nc.vector.tensor_copy(oAll[:, kt, 0], oLT2[:, kt])
            nc.vector.tensor_copy(oAll[:, kt, 1], oRT2[:, kt])

        # f16 copies of the replicated topi rows (for the hybrid mask path)
        tt0r = wk.tile([128, C], f16, tag="tt0r")
        tt1r = wk.tile([128, C], f16, tag="tt1r")
        nc.vector.tensor_copy(tt0r, ttrep_ps[0])
        nc.vector.tensor_copy(tt1r, ttrep_ps[1])

        # ---- replicate R.T rows across partitions via a DRAM round-trip ----
        dram = ctx.enter_context(tc.tile_pool(name="dram", bufs=1, space="DRAM"))
        rtd = dram.tile([C, C], f16)
        nc.gpsimd.dma_start(out=rtd[:], in_=RTh)
        rrep = wk.tile([128, C, C], f16, tag="rrep")
        rsrc = rtd[:]
        for ch in range(8):
            cs = slice(ch * 8, (ch + 1) * 8)
            src_ap = bass.AP(tensor=rsrc.tensor, offset=rsrc.offset + ch * 8 * C,
                             ap=[[0, 128], [C, 8], [1, C]])
            nc.gpsimd.dma_start(out=rrep[:, cs], in_=src_ap)

        # ---- expert stage: 4 experts per DVE mult, fp16 matmuls into PSUM ----
        ps_moe = ps_moe_p.tile([128, T], f32, tag="psmoe")
        nc.tensor.matmul(ps_moe[0:C], RTh, beh, start=True, stop=False,
                         skip_group_check=True)
        nc.tensor.matmul(ps_moe[C:128], RTh, beh, start=True, stop=False,
                         skip_group_check=True)
        # hybrid: first HYB experts via inline DVE masks, overlapping the
        # rrep DRAM round-trip; the rest use the replicated rrep rows.
        HYB = 16
        for e in range(HYB):
            m0 = msk_p.tile([128, C], f16, tag="m0")
            m1 = msk_p.tile([128, C], f16, tag="m1")
            nc.vector.tensor_scalar(out=m0, in0=tt0r, scalar1=float(e),
                                    scalar2=None, op0=OP.is_equal)
            nc.vector.tensor_scalar(out=m1, in0=tt1r, scalar1=float(e),
                                    scalar2=None, op0=OP.is_equal)
            nc.vector.tensor_add(m0, m0, m1)
            asch = asc_p.tile([128, KT, 2, C], f16, tag="asch")
            m0b = bass.AP(tensor=m0.tensor, offset=m0.offset,
                          ap=[list(m0.ap[0]), [0, KT], [0, 2], list(m0.ap[1])])
            nc.vector.tensor_tensor(out=asch, in0=oAll, in1=m0b, op=OP.mult)
            for kt in range(KT):
                nc.tensor.matmul(ps_moe, asch[:, kt], we_sb[:, e, kt],
                                 start=False, stop=False,
                                 skip_group_check=True)
        EG = 4
        for e0 in range(HYB, C, EG):
            asc = asc_p.tile([128, EG, KT, 2, C], f16, tag="asc")
            out_ap = bass.AP(tensor=asc.tensor, offset=asc.offset,
                             ap=[list(asc.ap[0]), [KT * 2 * C, EG], [1, KT * 2 * C]])
            in0 = bass.AP(tensor=oAll.tensor, offset=oAll.offset,
                          ap=[list(oAll.ap[0]), [0, EG], [1, KT * 2 * C]])
            rs = rrep[:, e0:e0 + EG]
            in1 = bass.AP(tensor=rs.tensor, offset=rs.offset,
                          ap=[list(rs.ap[0]), list(rs.ap[1]), [0, KT * 2],
                              [1, C]])
            nc.vector.tensor_tensor(out=out_ap, in0=in0, in1=in1, op=OP.mult)
            for i in range(EG):
                for kt in range(KT):
                    nc.tensor.matmul(ps_moe, asc[:, i, kt], we_sb[:, e0 + i, kt],
                                     start=False,
                                     stop=(e0 + EG >= C and i == EG - 1 and kt == KT - 1),
                                     skip_group_check=True)

        # ---- final LN + residual (beta+resid precombined off critical path) ----
        obl = wk.tile([C, T], f32, tag="obl")
        obr = wk.tile([C, T], f32, tag="obr")
        nc.vector.tensor_tensor(out=obl, in0=OUT_L, in1=vec["mbl"], op=OP.add)
        nc.vector.tensor_tensor(out=obr, in0=OUT_R, in1=vec["mbr"], op=OP.add)

        def ln2_block(src_ps, gamma, beta_resid, out_tile, stag):
            nc.vector.tensor_copy(out_tile, src_ps)
            stats = sm.tile([C, 6], f32, tag="stats" + stag)
            nc.vector.bn_stats(out=stats, in_=out_tile)
            mv = sm.tile([C, 2], f32, tag="mv" + stag)
            nc.vector.bn_aggr(out=mv, in_=stats)
            rstd = sm.tile([C, 1], f32, tag="rstd" + stag)
            nc.scalar.activation(out=rstd, in_=mv[:, 1:2], func=AF.Sqrt,
                                 bias=eps_t)
            nc.vector.reciprocal(rstd, rstd)
            nc.vector.tensor_scalar(out=out_tile, in0=out_tile,
                                    scalar1=mv[:, 0:1], scalar2=rstd,
                                    op0=OP.subtract, op1=OP.mult)
            nc.vector.tensor_tensor(out=out_tile, in0=out_tile, in1=gamma,
                                    op=OP.mult)
            nc.vector.tensor_tensor(out=out_tile, in0=out_tile, in1=beta_resid,
                                    op=OP.add)

        ol2 = wk.tile([C, T], f32, tag="ol2")
        or2 = wk.tile([C, T], f32, tag="or2")
        ln2_block(ps_moe[0:C], vec["mgl"], obl, ol2, "2l")
        ln2_block(ps_moe[C:128], vec["mgr"], obr, or2, "2r")
        nc.sync.dma_start(out=ol2_d.ap(), in_=ol2)
        nc.sync.dma_start(out=or2_d.ap(), in_=or2)

    nc.compile()
    return nc


def _tile_t(w):
    # (T_in, N) -> [128, T_in//128, N] partition-tiled
    t_in, n = w.shape
    return np.ascontiguousarray(w.reshape(t_in // 128, 128, n).transpose(1, 0, 2))


def _prep_in_maps(inputs):
    f = np.float32
    x_l, x_r = inputs["x_l"], inputs["x_r"]

    def rep(name):
        return np.repeat(np.asarray(inputs[name], f).reshape(1, T), C, axis=0)

    cen = np.asarray(inputs["centers"], f)
    cenn = cen / np.maximum(np.linalg.norm(cen, axis=-1, keepdims=True), 1e-12)
    sel = np.zeros((2, 2, 128), f)
    sel[0, 0, :] = 1.0
    sel[1, 1, :] = 1.0
    arrs = {
        "wqt": _tile_t(np.asarray(inputs["Wq"], f).T),
        "wkt": _tile_t(np.asarray(inputs["Wk"], f).T),
        "wvt": _tile_t(np.asarray(inputs["Wv"], f).T),
        "wpt": _tile_t(np.asarray(inputs["Wp"], f).T),
        "bqp": np.asarray(inputs["bq"], f).reshape(KT, 128).T,
        "bkp": np.asarray(inputs["bk"], f).reshape(KT, 128).T,
        "wrt": _tile_t(np.asarray(inputs["Wr"], f).T),
        "brp": np.asarray(inputs["br"], f).reshape(EXP, 1),
        "cent": np.ascontiguousarray(cenn.T),
        "ident": np.eye(128, dtype=f),
        "eiota": np.arange(C, dtype=f).reshape(C, 1),
        "sel": sel,
        "bv": rep("bv"), "bp": rep("bp"),
        "agl": rep("ag_l"), "abl": rep("ab_l"),
        "agr": rep("ag_r"), "abr": rep("ab_r"),
        "mgl": rep("mg_l"), "mbl": rep("mb_l"),
        "mgr": rep("mg_r"), "mbr": rep("mb_r"),
        "xl": np.zeros((C, T), f), "xr": np.zeros((C, T), f),
    }
    We = np.asarray(inputs["We"], f)
    WeTh = np.ascontiguousarray(
        We.transpose(0, 2, 1).reshape(C, KT, 128, T).transpose(2, 0, 1, 3)
    ).astype(np.float16)
    beh = np.asarray(inputs["be"], f).astype(np.float16)

    def pack(spec, ncols, extra):
        blob = np.zeros((128, ncols), f)
        for name, parts, shape in spec:
            off, _, _ = BLOB_OFF[name]
            cols = int(np.prod(shape[1:]))
            a = extra[name] if name in extra else arrs[name]
            blob[0:parts, off:off + cols] = np.asarray(a, f).reshape(parts, cols)
        return blob

    blobB = pack(BLOB_B_SPEC, NB_COLS, {})
    in_maps = []
    for b in range(N_CORES):
        xtl = _tile_t(np.ascontiguousarray(np.asarray(x_l[b], f).T))
        xtr = _tile_t(np.ascontiguousarray(np.asarray(x_r[b], f).T))
        blobA = pack(BLOB_A_SPEC, NA_COLS, {"xtl": xtl, "xtr": xtr})
        bB = blobB.copy()
        o, p, sh = BLOB_OFF["xl"]
        bB[0:p, o:o + T] = np.asarray(x_l[b], f)
        o, p, sh = BLOB_OFF["xr"]
        bB[0:p, o:o + T] = np.asarray(x_r[b], f)
        in_maps.append({"blobA": blobA, "blobB": bB, "weh": WeTh, "beh": beh})
    return in_maps


def kernel(**inputs) -> np.ndarray:
    from concourse.bass_utils import run_bass_kernel_spmd

    if "nc" not in _CACHE:
        _CACHE["nc"] = _build()
    nc = _CACHE["nc"]
    in_maps = _prep_in_maps(inputs)
    res = run_bass_kernel_spmd(nc, in_maps, list(range(N_CORES)))
    _CACHE["exec_time_ns"] = res.exec_time_ns
    out_l2 = np.stack([res.results[b]["ol2"] for b in range(N_CORES)])
    out_r2 = np.stack([res.results[b]["or2"] for b in range(N_CORES)])
    return np.stack([out_l2, out_r2]).astype(np.float32)



# revision 3
# speedup vs baseline: 1.0713x; 1.0713x over previous
"""Trainium2 Bass kernel for nn_BiDGNBlock (moe_routing).

Strategy: data-parallel over batch across 8 NeuronCores (no collectives).
Each core computes one batch element end-to-end.

Key optimizations vs the 70.4us baseline:
  - Expert table We streamed as fp8e4 (x128 scale; the final layer_norm is
    scale-invariant so the scale never needs to be divided out) -- halves
    the dominant 8.4MB DMA stream. Activations stay fp16: measured rel err
    1.56e-2 < 2e-2 gate, with exact-fp32 routing (picks verified identical).
  - Router norm chain deleted: top-k indices are invariant to the positive
    per-row scale 1/||xp||, and the top-2 softmax gate is exactly 1.0.
  - Softmax without row-max (|energy/16| < 0.75), exp fused with the 1/16
    scale in one scalar-engine activation.
  - LayerNorms restructured: proj bias preloaded into PSUM, bn_stats reads
    PSUM directly, normalize runs on the scalar engine (Identity with
    per-row scale/bias), beta+residual precombined host-side.
  - Expert masks: first 16 experts' masks built inline on DVE; the other 48
    replicated across partitions via a 2-descriptor DRAM round-trip on the
    otherwise-empty Activation-engine HWDGE queue (the baseline's 8 serial
    SWDGE descriptors + scheduler head-of-line block cost ~6us).
  - PE kept warm (HAM k=8/8) with dummy matmuls through the attention
    phase and a burst before the expert phase (HAM drops to half clock
    after ~2us of low PE duty; the baseline ran experts at k=4 for 13us).
  - Single output DMA on the empty Activation queue (baseline's outputs
    queued behind the We stream on the sync queue).
"""

import sys
import numpy as np

sys.path.insert(0, "/opt/trn_rl_repo")

N_CORES = 8
B, C, T = 8, 64, 256
EXP = 32
KT = T // 128  # 2 k-tiles over the feature dim
WE_SCALE = 128.0

_CACHE: dict = {}

# fp32 blob layouts: (name, partitions, shape). cols = prod(shape[1:]).
BLOB_A_SPEC = [
    ("xtl", 128, (128, KT, C)), ("xtr", 128, (128, KT, C)),
    ("wqt", 128, (128, KT, T)), ("wkt", 128, (128, KT, T)),
    ("bqp", 128, (128, KT)), ("bkp", 128, (128, KT)),
]
BLOB_B1_SPEC = [
    ("wvt", 128, (128, KT, T)), ("wpt", 128, (128, KT, T)),
    ("wrt", 128, (128, 2 * KT, EXP)),
    ("ident", 64, (64, 64)), ("sel", 2, (2, 2, 128)),
    ("xlb", 64, (64, T)), ("xrb", 64, (64, T)),
    ("bvr", 64, (64, T)), ("bpr", 64, (64, T)),
    ("aglr", 64, (64, T)), ("agrr", 64, (64, T)),
]
BLOB_B2_SPEC = [
    ("e8", 128, (128, C)),
    ("mglr", 64, (64, T)), ("mgrr", 64, (64, T)),
    ("oblb", 64, (64, T)), ("obrb", 64, (64, T)),
    ("behs", 64, (64, T)),
    ("brp", 32, (32, 1)), ("cent", 32, (32, C)), ("eiota", 64, (64, 1)),
]


def _layout(spec):
    off = {}
    n = 0
    for name, parts, shape in spec:
        cols = int(np.prod(shape[1:]))
        off[name] = (n, parts, shape)
        n += cols
    return off, n


OFF_A, NA = _layout(BLOB_A_SPEC)
OFF_B1, NB1 = _layout(BLOB_B1_SPEC)
OFF_B2, NB2 = _layout(BLOB_B2_SPEC)
BLOB_OFF = {**OFF_A, **OFF_B1, **OFF_B2}


def _build():
    import concourse.bass as bass
    import concourse.mybir as mybir
    import concourse.tile as tile
    from concourse import bacc
    from contextlib import ExitStack

    dt = mybir.dt
    f32, f16, f8 = dt.float32, dt.float16, dt.float8e4
    AF = mybir.ActivationFunctionType
    OP = mybir.AluOpType

    nc = bacc.Bacc("TRN2", target_bir_lowering=False, debug=False,
                   num_devices=N_CORES)

    blobA_d = nc.dram_tensor("blobA", [128, NA], f32, kind="ExternalInput")
    blobB1_d = nc.dram_tensor("blobB1", [128, NB1], f32, kind="ExternalInput")
    blobB2_d = nc.dram_tensor("blobB2", [128, NB2], f32, kind="ExternalInput")
    weq_d = nc.dram_tensor("weq", [128, C, KT, T], f8, kind="ExternalInput")
    oboth_d = nc.dram_tensor("oboth", [C, 2 * T], f32, kind="ExternalOutput")

    with tile.TileContext(nc) as tc, ExitStack() as ctx:
        cst = ctx.enter_context(tc.tile_pool(name="cst", bufs=1))
        wk = ctx.enter_context(tc.tile_pool(name="wk", bufs=2))
        sm = ctx.enter_context(tc.tile_pool(name="sm", bufs=2))
        msk_p = ctx.enter_context(tc.tile_pool(name="msk", bufs=2))
        asc_p = ctx.enter_context(tc.tile_pool(name="asc", bufs=3))
        ps = ctx.enter_context(tc.tile_pool(name="ps", bufs=2, space="PSUM"))
        proj_p = ctx.enter_context(tc.tile_pool(name="proj", bufs=2, space="PSUM"))
        moe_p = ctx.enter_context(tc.tile_pool(name="moe", bufs=1, space="PSUM"))
        warm_p = ctx.enter_context(tc.tile_pool(name="warm", bufs=1, space="PSUM"))
        dram = ctx.enter_context(tc.tile_pool(name="dram", bufs=1, space="DRAM"))

        # ---- input DMAs: sync (SP) HWDGE queue, FIFO priority order ----
        blobA = cst.tile([128, NA], f32, tag="blobA")
        nc.sync.dma_start(out=blobA, in_=blobA_d.ap())
        blobB1 = cst.tile([128, NB1], f32, tag="blobB1")
        nc.sync.dma_start(out=blobB1, in_=blobB1_d.ap())
        blobB2 = cst.tile([128, NB2], f32, tag="blobB2")
        nc.sync.dma_start(out=blobB2, in_=blobB2_d.ap())
        weq = cst.tile([128, C, KT, T], f8, tag="weq")
        wea = weq_d.ap()
        for ch in range(8):
            nc.sync.dma_start(out=weq[:, ch * 8:(ch + 1) * 8],
                              in_=wea[:, ch * 8:(ch + 1) * 8])

        def bview(blob, name):
            off, parts, shape = BLOB_OFF[name]
            cols = int(np.prod(shape[1:]))
            v = blob[0:parts, off:off + cols]
            if len(shape) == 3:
                v = v.rearrange("p (a b) -> p a b", a=shape[1])
            return v

        xtl = bview(blobA, "xtl")
        xtr = bview(blobA, "xtr")
        wqt = bview(blobA, "wqt")
        wkt = bview(blobA, "wkt")
        bqp = bview(blobA, "bqp")
        bkp = bview(blobA, "bkp")
        wvt = bview(blobB1, "wvt")
        wpt = bview(blobB1, "wpt")
        wrt = bview(blobB1, "wrt")
        ident = bview(blobB1, "ident")
        sel = bview(blobB1, "sel")
        xlb = bview(blobB1, "xlb")
        xrb = bview(blobB1, "xrb")
        bvr = bview(blobB1, "bvr")
        bpr = bview(blobB1, "bpr")
        aglr = bview(blobB1, "aglr")
        agrr = bview(blobB1, "agrr")
        e8 = bview(blobB2, "e8")
        mglr = bview(blobB2, "mglr")
        mgrr = bview(blobB2, "mgrr")
        oblb = bview(blobB2, "oblb")
        obrb = bview(blobB2, "obrb")
        behs = bview(blobB2, "behs")
        brp = bview(blobB2, "brp")
        cent = bview(blobB2, "cent")
        eiota = bview(blobB2, "eiota")

        eps_t = cst.tile([C, 1], f32, tag="eps")
        nc.vector.memset(eps_t, 1e-5)

        # ---- PE warm-up from memset tiles + ACT table preloads ----
        wsrc = cst.tile([128, 512], f16, tag="wsrc")
        nc.vector.memset(wsrc, 0.5)
        pw = warm_p.tile([128, 512], f32, tag="warm")
        for wi in range(6):
            nc.tensor.matmul(pw, wsrc[:, 0:128], wsrc,
                             start=True, stop=True, skip_group_check=True)
        wact = cst.tile([1, 32], f32, tag="wact")
        nc.vector.memset(wact, 1.0)
        nc.scalar.activation(out=wact, in_=wact, func=AF.Exp)
        nc.scalar.activation(out=wact, in_=wact, func=AF.Sqrt)
        nc.scalar.activation(out=wact, in_=wact, func=AF.Identity)

        def dmy(n=1, cols=256):
            for _ in range(n):
                nc.tensor.matmul(pw[:, 0:cols], wsrc[:, 0:128], wsrc[:, 0:cols],
                                 start=True, stop=True, skip_group_check=True)

        # ---- off-critical-path casts on gpsimd (idle early) ----
        e8h = cst.tile([128, C], f16, tag="e8h")
        nc.gpsimd.tensor_copy(e8h, e8)
        behh = cst.tile([C, T], f16, tag="behh")
        nc.gpsimd.tensor_copy(behh, behs)

        # ---- attention: q.T, k.T ----
        qt = wk.tile([128, KT, C], f32, tag="qt")
        ktl = wk.tile([128, KT, C], f32, tag="ktl")
        for (src, w, bias, dst) in [(xtl, wqt, bqp, qt), (xtr, wkt, bkp, ktl)]:
            for ut in range(KT):
                p = ps.tile([128, C], f32, tag="ps")
                for kt in range(KT):
                    nc.tensor.matmul(p, w[:, kt, ut * 128:(ut + 1) * 128],
                                     src[:, kt], start=(kt == 0), stop=(kt == KT - 1))
                nc.vector.tensor_scalar(out=dst[:, ut], in0=p,
                                        scalar1=bias[:, ut:ut + 1], scalar2=None,
                                        op0=OP.add)

        # ---- v = (x_l - x_r) @ Wv.T + bv  (natural layout [c, u]) ----
        xdt = wk.tile([128, KT, C], f32, tag="xdt")
        nc.vector.tensor_sub(xdt, xtl, xtr)
        pv = ps.tile([C, T], f32, tag="ps")
        for kt in range(KT):
            nc.tensor.matmul(pv, xdt[:, kt], wvt[:, kt],
                             start=(kt == 0), stop=(kt == KT - 1))
        v_sb = wk.tile([C, T], f32, tag="v")
        nc.vector.tensor_tensor(out=v_sb, in0=pv, in1=bvr, op=OP.add)

        # ---- energy -> exp(energy/16) directly (|arg| < 0.75, no row-max) ----
        pe_ = ps.tile([C, C], f32, tag="ps")
        for ut in range(KT):
            nc.tensor.matmul(pe_, qt[:, ut], ktl[:, ut],
                             start=(ut == 0), stop=(ut == KT - 1))
        dmy(2)
        attn = wk.tile([C, C], f32, tag="attn")
        nc.scalar.activation(out=attn, in_=pe_, func=AF.Exp, scale=1.0 / 16.0)
        rowsum = sm.tile([C, 1], f32, tag="rowsum")
        nc.vector.tensor_reduce(rowsum, attn, axis=mybir.AxisListType.X, op=OP.add)
        nc.vector.reciprocal(rowsum, rowsum)
        nc.vector.tensor_scalar_mul(attn, attn, rowsum)

        # ---- attn.T ----
        pat = ps.tile([C, C], f32, tag="ps")
        nc.tensor.transpose(pat, attn, ident)
        attnT = wk.tile([C, C], f32, tag="attnT")
        nc.vector.tensor_copy(attnT, pat)

        # ---- proj psum bias pre-init (off critical path, DVE idle here) ----
        ppl = proj_p.tile([C, T], f32, tag="projl")
        ppr = proj_p.tile([C, T], f32, tag="projr")
        nc.vector.tensor_copy(ppl, bpr)
        nc.vector.tensor_copy(ppr, bpr)

        # ---- out_l.T / out_r.T  [u, c] ----
        oLT = wk.tile([128, KT, C], f32, tag="oLT")
        oRT = wk.tile([128, KT, C], f32, tag="oRT")
        for ut in range(KT):
            pl = ps.tile([128, C], f32, tag="ps")
            nc.tensor.matmul(pl, v_sb[:, ut * 128:(ut + 1) * 128], attnT,
                             start=True, stop=True)
            nc.vector.tensor_copy(oLT[:, ut], pl)
            pr = ps.tile([128, C], f32, tag="ps")
            nc.tensor.matmul(pr, v_sb[:, ut * 128:(ut + 1) * 128], attn,
                             start=True, stop=True)
            nc.scalar.copy(oRT[:, ut], pr)
        dmy(2)

        # ---- proj (accumulates onto pre-loaded bias) ----
        for ut in range(KT):
            nc.tensor.matmul(ppl, oLT[:, ut], wpt[:, ut],
                             start=False, stop=(ut == KT - 1))
        for ut in range(KT):
            nc.tensor.matmul(ppr, oRT[:, ut], wpt[:, ut],
                             start=False, stop=(ut == KT - 1))

        # ---- LN1 both sides (stats straight off PSUM; scalar-engine
        #      normalize; beta+residual precombined host-side in xlb/xrb) ----
        def ln_start(src_ps, stag):
            stats = sm.tile([C, 6], f32, tag="st" + stag)
            nc.vector.bn_stats(out=stats, in_=src_ps)
            mv = sm.tile([C, 2], f32, tag="mv" + stag)
            nc.vector.bn_aggr(out=mv, in_=stats)
            rstd = sm.tile([C, 1], f32, tag="rs" + stag)
            nc.scalar.activation(out=rstd, in_=mv[:, 1:2], func=AF.Sqrt,
                                 bias=eps_t)
            nc.vector.reciprocal(rstd, rstd)
            nmr = sm.tile([C, 1], f32, tag="nm" + stag)
            nc.vector.tensor_scalar(out=nmr, in0=mv[:, 0:1], scalar1=rstd,
                                    scalar2=-1.0, op0=OP.mult, op1=OP.mult)
            return rstd, nmr

        def ln_finish(src_ps, rstd, nmr, gamma, betaresid, out_tile, stag):
            nrm = sm.tile([C, T], f32, tag="nr" + stag)
            nc.scalar.activation(out=nrm, in_=src_ps, func=AF.Identity,
                                 bias=nmr, scale=rstd)
            nc.vector.tensor_tensor(out=out_tile, in0=nrm, in1=gamma,
                                    op=OP.mult)
            nc.vector.tensor_tensor(out=out_tile, in0=out_tile, in1=betaresid,
                                    op=OP.add)

        OUT_L = wk.tile([C, T], f32, tag="OUTL")
        OUT_R = wk.tile([C, T], f32, tag="OUTR")
        rl, nl = ln_start(ppl, "1l")
        rr, nr_ = ln_start(ppr, "1r")
        ln_finish(ppl, rl, nl, aglr, xlb, OUT_L, "1l")
        ln_finish(ppr, rr, nr_, agrr, xrb, OUT_R, "1r")
        dmy(2)

        # ---- transposes -> [u(128), kt, c] f32 (router) + f16 oAll (experts) ----
        oLT2 = wk.tile([128, KT, C], f32, tag="oLT2")
        oRT2 = wk.tile([128, KT, C], f32, tag="oRT2")
        oAll = wk.tile([128, KT, 2, C], f16, tag="oAll")
        for (side, src, dst) in [(0, OUT_L, oLT2), (1, OUT_R, oRT2)]:
            for ut in range(KT):
                pt = ps.tile([128, C], f32, tag="ps")
                nc.tensor.transpose(pt, src[:, ut * 128:(ut + 1) * 128], ident)
                nc.vector.tensor_copy(dst[:, ut], pt)
                nc.scalar.copy(oAll[:, ut, side], pt)

        # ---- router: xp.T then sims (no norms: top-k is scale-invariant
        #      per row and the top-2 softmax gate is exactly 1.0) ----
        rtiles = [(oLT2, 0), (oLT2, 1), (oRT2, 0), (oRT2, 1)]
        pxp = ps.tile([EXP, C], f32, tag="ps")
        for j, (tl, kt) in enumerate(rtiles):
            nc.tensor.matmul(pxp, wrt[:, j], tl[:, kt],
                             start=(j == 0), stop=(j == 3))
        xpT = sm.tile([EXP, C], f32, tag="xpT")
        nc.vector.tensor_scalar(out=xpT, in0=pxp, scalar1=brp, scalar2=None,
                                op0=OP.add)
        psim = ps.tile([C, C], f32, tag="ps")
        nc.tensor.matmul(psim, xpT, cent, start=True, stop=True)
        sim_sb = sm.tile([C, C], f32, tag="sim")
        nc.scalar.copy(sim_sb, psim)
        dmy(10, 512)  # re-ramp HAM before the expert phase

        mx8 = sm.tile([C, 8], f32, tag="mx8")
        nc.vector.max(out=mx8, in_=sim_sb)
        idx8 = sm.tile([C, 8], mybir.dt.uint32, tag="idx8")
        nc.vector.max_index(out=idx8, in_max=mx8, in_values=sim_sb)
        topif = sm.tile([C, 2], f32, tag="topif")
        nc.vector.tensor_copy(topif, idx8[:, 0:2])

        # ---- replicate topi rows across partitions via PE ----
        ptt = ps.tile([2, C], f32, tag="ps")
        nc.tensor.transpose(ptt, topif, ident)
        ttT = sm.tile([2, C], f32, tag="ttT")
        nc.vector.tensor_copy(ttT, ptt)
        pr0 = ps.tile([128, C], f32, tag="ps")
        nc.tensor.matmul(pr0, sel[:, 0], ttT, start=True, stop=True)
        pr1 = ps.tile([128, C], f32, tag="ps")
        nc.tensor.matmul(pr1, sel[:, 1], ttT, start=True, stop=True)
        tt0r = wk.tile([128, C], f16, tag="tt0r")
        tt1r = wk.tile([128, C], f16, tag="tt1r")
        nc.vector.tensor_copy(tt0r, pr0)
        nc.scalar.copy(tt1r, pr1)

        # ---- R.T[e, c] (f16) for the bias matmul + partition round-trip ----
        RT0 = sm.tile([C, C], f16, tag="RT0")
        RT1 = sm.tile([C, C], f16, tag="RT1")
        nc.vector.tensor_scalar(out=RT0, in0=pr0[0:C], scalar1=eiota,
                                scalar2=None, op0=OP.is_equal)
        nc.vector.tensor_scalar(out=RT1, in0=pr1[0:C], scalar1=eiota,
                                scalar2=None, op0=OP.is_equal)
        RThh = wk.tile([C, C], f16, tag="RThh")
        nc.vector.tensor_add(RThh, RT0, RT1)

        # experts 16..63 masks: DRAM round-trip on the (empty) Activation
        # HWDGE queue -- one write + one broadcast-read descriptor pair.
        rtd = dram.tile([C, C], f16)
        nc.scalar.dma_start(out=rtd[:], in_=RThh)
        mrep1 = wk.tile([128, C - 16, C], f16, tag="mrep1")
        rsrc = rtd[:]
        src_ap = bass.AP(tensor=rsrc.tensor, offset=rsrc.offset + 16 * C,
                         ap=[[0, 128], [C, C - 16], [1, C]])
        nc.scalar.dma_start(out=mrep1, in_=src_ap)

        # ---- expert stage ----
        ps_moe = moe_p.tile([128, T], f32, tag="psmoe")
        nc.tensor.matmul(ps_moe[0:C], RThh, behh, start=True, stop=False,
                         skip_group_check=True)
        nc.tensor.matmul(ps_moe[C:128], RThh, behh, start=True, stop=False,
                         skip_group_check=True)

        EG = 4
        mrep0 = wk.tile([128, 16, C], f16, tag="mrep0")

        def asch_mult(dst, msrc_ap):
            # dst[p, e(EG), kt, side, c] = oAll[p, kt, side, c] * m[e, c]
            out_ap = bass.AP(tensor=dst.tensor, offset=dst.offset,
                             ap=[list(dst.ap[0]), [KT * 2 * C, EG],
                                 [1, KT * 2 * C]])
            in0 = bass.AP(tensor=oAll.tensor, offset=oAll.offset,
                          ap=[list(oAll.ap[0]), [0, EG], [1, KT * 2 * C]])
            nc.vector.tensor_tensor(out=out_ap, in0=in0, in1=msrc_ap, op=OP.mult)

        def mask_bcast_ap(mt, col0):
            # [p, e(EG), kt*side(bcast), c] view of a [128, ncols, C] tile
            return bass.AP(tensor=mt.tensor, offset=mt.offset + col0 * C,
                           ap=[list(mt.ap[0]), [C, EG], [0, KT * 2], [1, C]])

        for g in range(C // EG):
            e0 = g * EG
            if e0 < 16:
                # inline masks for experts 0..15 (cover the round-trip)
                m0 = msk_p.tile([128, EG, C], f16, tag="m0")
                in0a = bass.AP(tensor=tt0r.tensor, offset=tt0r.offset,
                               ap=[list(tt0r.ap[0]), [0, EG], [1, C]])
                in1a = bass.AP(tensor=e8h.tensor, offset=e8h.offset + e0,
                               ap=[list(e8h.ap[0]), [1, EG], [0, C]])
                nc.vector.tensor_tensor(out=m0, in0=in0a, in1=in1a,
                                        op=OP.is_equal)
                m1 = msk_p.tile([128, EG, C], f16, tag="m1")
                in0b = bass.AP(tensor=tt1r.tensor, offset=tt1r.offset,
                               ap=[list(tt1r.ap[0]), [0, EG], [1, C]])
                nc.vector.tensor_tensor(out=m1, in0=in0b, in1=in1a,
                                        op=OP.is_equal)
                nc.vector.tensor_add(mrep0[:, e0:e0 + EG], m0, m1)
                msrc = mask_bcast_ap(mrep0, e0)
            else:
                msrc = mask_bcast_ap(mrep1, e0 - 16)
            asch = asc_p.tile([128, EG, KT, 2, C], f16, tag="asc")
            asch_mult(asch, msrc)
            for i in range(EG):
                for kt in range(KT):
                    nc.tensor.matmul(
                        ps_moe, asch[:, i, kt], weq[:, e0 + i, kt],
                        start=False,
                        stop=(e0 + EG >= C and i == EG - 1 and kt == KT - 1),
                        skip_group_check=True)

        # ---- beta+residual for LN2 on gpsimd during the expert phase ----
        obl = wk.tile([C, T], f32, tag="obl")
        obr = wk.tile([C, T], f32, tag="obr")
        nc.gpsimd.tensor_add(obl, OUT_L, oblb)
        nc.gpsimd.tensor_add(obr, OUT_R, obrb)

        # ---- final LN + residual -> oboth, single DMA out ----
        oboth = wk.tile([C, 2 * T], f32, tag="oboth")
        r2l, n2l = ln_start(ps_moe[0:C], "2l")
        r2r, n2r = ln_start(ps_moe[C:128], "2r")
        ln_finish(ps_moe[0:C], r2l, n2l, mglr, obl, oboth[:, 0:T], "2l")
        ln_finish(ps_moe[C:128], r2r, n2r, mgrr, obr, oboth[:, T:2 * T], "2r")
        nc.scalar.dma_start(out=oboth_d.ap(), in_=oboth)

    nc.compile()
    return nc


def _tile_t(w):
    # (T_in, N) -> [128, T_in//128, N] partition-tiled
    t_in, n = w.shape
    return np.ascontiguousarray(w.reshape(t_in // 128, 128, n).transpose(1, 0, 2))


def _prep_in_maps(inputs):
    import ml_dtypes

    f = np.float32
    x_l, x_r = inputs["x_l"], inputs["x_r"]

    def rep(v):
        return np.repeat(np.asarray(v, f).reshape(1, T), C, axis=0)

    cen = np.asarray(inputs["centers"], f)
    cenn = cen / np.maximum(np.linalg.norm(cen, axis=-1, keepdims=True), 1e-12)
    sel = np.zeros((2, 2, 128), f)
    sel[0, 0, :] = 1.0
    sel[1, 1, :] = 1.0
    arrs = {
        "wqt": _tile_t(np.asarray(inputs["Wq"], f).T),
        "wkt": _tile_t(np.asarray(inputs["Wk"], f).T),
        "wvt": _tile_t(np.asarray(inputs["Wv"], f).T),
        "wpt": _tile_t(np.asarray(inputs["Wp"], f).T),
        "bqp": np.asarray(inputs["bq"], f).reshape(KT, 128).T,
        "bkp": np.asarray(inputs["bk"], f).reshape(KT, 128).T,
        "wrt": _tile_t(np.asarray(inputs["Wr"], f).T),
        "brp": np.asarray(inputs["br"], f).reshape(EXP, 1),
        "cent": np.ascontiguousarray(cenn.T),
        "ident": np.eye(64, dtype=f),
        "eiota": np.arange(C, dtype=f).reshape(C, 1),
        "e8": np.tile(np.arange(C, dtype=f), (128, 1)),
        "sel": sel,
        "bvr": rep(inputs["bv"]), "bpr": rep(inputs["bp"]),
        "aglr": rep(inputs["ag_l"]), "agrr": rep(inputs["ag_r"]),
        "mglr": rep(inputs["mg_l"]), "mgrr": rep(inputs["mg_r"]),
        "oblb": rep(inputs["mb_l"]), "obrb": rep(inputs["mb_r"]),
        "behs": np.asarray(inputs["be"], f) * WE_SCALE,
        "xlb": np.zeros((C, T), f), "xrb": np.zeros((C, T), f),
    }
    # We -> [128(t%128), C, KT, T(u)] fp8e4, x128 (layer_norm absorbs it)
    We = np.asarray(inputs["We"], f)
    WeT = We.transpose(0, 2, 1).reshape(C, KT, 128, T).transpose(2, 0, 1, 3)
    weq = np.ascontiguousarray(WeT * WE_SCALE).astype(ml_dtypes.float8_e4m3)

    def pack(spec, ncols, extra):
        blob = np.zeros((128, ncols), f)
        for name, parts, shape in spec:
            off, _, _ = BLOB_OFF[name]
            cols = int(np.prod(shape[1:]))
            a = extra[name] if name in extra else arrs[name]
            blob[0:parts, off:off + cols] = np.asarray(a, f).reshape(parts, cols)
        return blob

    blobB2 = pack(BLOB_B2_SPEC, NB2, {})
    abl = np.asarray(inputs["ab_l"], f).reshape(1, T)
    abr = np.asarray(inputs["ab_r"], f).reshape(1, T)
    in_maps = []
    for b in range(N_CORES):
        xtl = _tile_t(np.ascontiguousarray(np.asarray(x_l[b], f).T))
        xtr = _tile_t(np.ascontiguousarray(np.asarray(x_r[b], f).T))
        blobA = pack(BLOB_A_SPEC, NA, {"xtl": xtl, "xtr": xtr})
        blobB1 = pack(BLOB_B1_SPEC, NB1,
                      {"xlb": np.asarray(x_l[b], f) + abl,
                       "xrb": np.asarray(x_r[b], f) + abr})
        in_maps.append({"blobA": blobA, "blobB1": blobB1, "blobB2": blobB2,
                        "weq": weq})
    return in_maps


def kernel(**inputs) -> np.ndarray:
    from concourse.bass_utils import run_bass_kernel_spmd

    if "nc" not in _CACHE:
        _CACHE["nc"] = _build()
    nc = _CACHE["nc"]
    in_maps = _prep_in_maps(inputs)
    res = run_bass_kernel_spmd(nc, in_maps, list(range(N_CORES)))
    _CACHE["exec_time_ns"] = res.exec_time_ns
    both = np.stack([res.results[b]["oboth"] for b in range(N_CORES)])
    out_l2 = both[:, :, 0:T]
    out_r2 = both[:, :, T:2 * T]
    return np.stack([out_l2, out_r2]).astype(np.float32)
